# revision 1
# baseline (speedup 1.0000x reference)
"""Channel-attention (nn_ChannelAttentionModule) Trainium2 kernel.

Math (per batch b):
    X = x[b]  [C, N]  with C=512, N=64*64=4096
    q = Wq X + bq ; k = Wk X + bk ; v = Wv X + bv
    L = q k^T                       [C, C]
    out = softmax(L, -1) v + X      [C, N]

Key restructure: L = Wq G Wk^T + bq (Wk S + N bk)^T + (Wq S) bk^T  (outer
products), where G = X X^T (Gram, symmetric) and S = X 1 (row sums).
G is computed in a single fp16 pass (~11-bit input mantissa, 1 cyc/row
on the PE, fp32 PSUM accumulation) over the upper block-triangle,
mirrored via PE transposes; the two 512^3 projection matmuls run in
true fp32; the v-path runs in fp16.  Softmax logits stay fp32.

Sharding: pure data-parallel, one batch per NeuronCore (B=8, 8 cores).
"""

import numpy as np

import concourse.mybir as mybir
import concourse.tile as tile
from concourse import bacc
from concourse.bass_utils import run_bass_kernel_spmd

F32 = mybir.dt.float32
F32R = mybir.dt.float32r
F16 = mybir.dt.float16
AX = mybir.AxisListType.X
EXP = mybir.ActivationFunctionType.Exp

B = 8
C = 512
HW = 64 * 64
P = 128
CH = C // P  # 4 channel chunks
NT = HW // 512  # 8 spatial tiles of 512
NG = 8  # xtr granules (4 spatial chunks each)
# upper-triangle start per G row chunk
USTART = [0, 128, 256, 256]


def _body(tc, nc, io):
    xt16, x16 = io["xt16"], io["x16"]
    wqh, wql, wkh, wkl, wvt = io["wqh"], io["wql"], io["wkh"], io["wkl"], io["wvt"]
    bqr, bkr, nbkr, bvc = io["bqr"], io["bkr"], io["nbkr"], io["bvc"]
    id16, out = io["id16"], io["out"]

    ps = tc.alloc_tile_pool(name="ps", bufs=1, space="PSUM")
    sb = tc.alloc_tile_pool(name="sb", bufs=1)
    st = tc.alloc_tile_pool(name="st", bufs=3)
    so = tc.alloc_tile_pool(name="so", bufs=2)

    wv_sb = sb.tile([P, CH * C], F16, name="wv_sb", tag="wv_sb")
    bv_sb = sb.tile([P, CH], F32, name="bv_sb", tag="bv_sb")
    x16_sb = [sb.tile([P, HW], F16, name=f"x16_{i}", tag=f"x16_{i}") for i in range(CH)]
    v_sb = [sb.tile([P, HW], F16, name=f"vsb{i}", tag=f"vsb{i}") for i in range(CH)]
    wqh_sb = sb.tile([P, CH * C], F16, name="wqh_sb", tag="wqh_sb")
    wql_sb = sb.tile([P, CH * C], F16, name="wql_sb", tag="wql_sb")
    wkh_sb = sb.tile([P, CH * C], F16, name="wkh_sb", tag="wkh_sb")
    wkl_sb = sb.tile([P, CH * C], F16, name="wkl_sb", tag="wkl_sb")

    def wslice(tile_, e, lo, hi):
        return tile_[:, e * C + lo : e * C + hi]

    def v_conv(nt, tag):
        for o in range(CH):
            v_ps = ps.tile([P, 512], F32, name=f"vps{o}", tag=f"{tag}{o}")
            for c in range(CH):
                nc.tensor.matmul(
                    v_ps,
                    lhsT=wslice(wv_sb, c, o * P, (o + 1) * P),
                    rhs=x16_sb[c][:, nt * 512 : (nt + 1) * 512],
                    start=c == 0,
                    stop=c == CH - 1,
                )
            nc.vector.tensor_scalar_add(
                v_sb[o][:, nt * 512 : (nt + 1) * 512], v_ps, bv_sb[:, o : o + 1]
            )

    # ---- interleaved front: x16/xtr stream + v-conv/G rounds ----
    ar_sb = [
        sb.tile([P, 4 * C], F16, name=f"ar{g}", tag=f"ar{g}") for g in range(NG)
    ]
    xtr3 = xt16.rearrange("(g t p) c -> g p t c", p=P, t=4)
    g_ps = [ps.tile([P, C], F32, name=f"gps{i}", tag=f"pa{i}") for i in range(CH)]

    def x16_load(nt2):
        for c in range(CH):
            nc.gpsimd.dma_start(
                x16_sb[c][:, nt2 * 1024 : (nt2 + 1) * 1024],
                x16[c * P : (c + 1) * P, nt2 * 1024 : (nt2 + 1) * 1024],
            )

    def xtr_load(g2):
        nc.sync.dma_start(ar_sb[g2].rearrange("p (t c) -> p t c", t=4), xtr3[g2])

    def g_pass(g2):
        ar4 = ar_sb[g2]
        for t in range(4):
            n = g2 * 4 + t
            first, last = n == 0, n == 4 * NG - 1
            for c in range(CH):
                u = USTART[c]
                nc.tensor.matmul(
                    g_ps[c][:, u:],
                    lhsT=ar4[:, t * C + c * P : t * C + (c + 1) * P],
                    rhs=ar4[:, t * C + u : (t + 1) * C],
                    start=first,
                    stop=last,
                )

    nc.sync.dma_start(
        ar_sb[0][:, 0 : 2 * C].rearrange("p (t c) -> p t c", t=2), xtr3[0][:, 0:2]
    )
    nc.sync.dma_start(
        ar_sb[0][:, 2 * C :].rearrange("p (t c) -> p t c", t=2), xtr3[0][:, 2:4]
    )
    xtr_load(1)
    nc.sync.dma_start(
        wv_sb.rearrange("p (e c) -> p e c", e=CH),
        wvt.rearrange("(e p) c -> p e c", p=P),
    )
    nc.sync.dma_start(
        bv_sb.rearrange("p (e o) -> p e o", e=CH),
        bvc.rearrange("(e p) o -> p e o", p=P),
    )
    x16_load(0)
    g_pass(0)
    g_pass(1)
    xtr_load(2)
    xtr_load(3)
    x16_load(1)
    v_conv(0, "pb")
    v_conv(1, "pb")
    g_pass(2)
    g_pass(3)
    xtr_load(4)
    xtr_load(5)
    x16_load(2)
    v_conv(2, "pb")
    v_conv(3, "pb")
    g_pass(4)
    g_pass(5)
    xtr_load(6)
    xtr_load(7)
    for c in range(CH):
        nc.sync.dma_start(
            x16_sb[c][:, 3 * 1024 : 4 * 1024],
            x16[c * P : (c + 1) * P, 3 * 1024 : 4 * 1024],
        )
    for wtile, wdram in ((wqh_sb, wqh), (wkh_sb, wkh), (wql_sb, wql), (wkl_sb, wkl)):
        nc.sync.dma_start(
            wtile.rearrange("p (e c) -> p e c", e=CH),
            wdram.rearrange("(e p) c -> p e c", p=P),
        )
    v_conv(4, "pb")
    v_conv(5, "pb")
    s_col = [sb.tile([P, 1], F32, name=f"s{i}", tag=f"s{i}") for i in range(CH)]
    for i in range(CH):
        nc.vector.reduce_sum(s_col[i], x16_sb[i], axis=AX)
    g_pass(6)
    g_pass(7)
    v_conv(6, "pb")

    # ---- consts needed by the mid/late phases ----
    id16_sb = sb.tile([P, P], F16, name="id16sb", tag="id16sb")
    nc.sync.dma_start(id16_sb, id16)
    nbkr_sb = sb.tile([1, C], F32, name="nbkrsb", tag="nbkrsb")
    nc.sync.dma_start(nbkr_sb, nbkr)

    # ---- u1 = (Wq S)^T, u2 = (Wk S)^T (fp16-hi; error ~1e-4 on logits) ----
    s16 = [sb.tile([P, 1], F16, name=f"s16_{i}", tag=f"s16_{i}") for i in range(CH)]
    for i in range(CH):
        nc.scalar.copy(s16[i], s_col[i])
    u1_ps = ps.tile([1, C], F32, name="u1ps", tag="pb0")
    u2_ps = ps.tile([1, C], F32, name="u2ps", tag="pb1")
    for e in range(CH):
        nc.tensor.matmul(
            u1_ps, lhsT=s16[e], rhs=wslice(wqh_sb, e, 0, C),
            start=e == 0, stop=e == CH - 1,
        )
    for e in range(CH):
        nc.tensor.matmul(
            u2_ps, lhsT=s16[e], rhs=wslice(wkh_sb, e, 0, C),
            start=e == 0, stop=e == CH - 1,
        )

    # ---- G split straight from PSUM: gh = f16(G), gl = G - gh; lower
    #      blocks mirrored with f16 PE transposes (exact: transpose of the
    #      rounded equals rounding of the transpose for symmetric G) ----
    gh = [sb.tile([P, C], F16, name=f"gh{i}", tag=f"gh{i}") for i in range(CH)]
    gl = [sb.tile([P, C], F16, name=f"gl{i}", tag=f"gl{i}") for i in range(CH)]
    for c in range(CH):
        u = USTART[c]
        nc.scalar.copy(gh[c][:, u:], g_ps[c][:, u:])
        nc.vector.tensor_sub(gl[c][:, u:], g_ps[c][:, u:], gh[c][:, u:])
        for d in range(u // P):
            tbh = ps.tile([P, P], F16, name="tbh", tag=f"pb{2 + (c + d) % 2}")
            nc.tensor.transpose(tbh, gh[d][:, c * P : (c + 1) * P], id16_sb)
            nc.scalar.copy(gh[c][:, d * P : (d + 1) * P], tbh)
            tbl = ps.tile([P, P], F16, name="tbl", tag=f"pb{2 + (c + d + 1) % 2}")
            nc.tensor.transpose(tbl, gl[d][:, c * P : (c + 1) * P], id16_sb)
            nc.vector.tensor_copy(gl[c][:, d * P : (d + 1) * P], tbl)

    u1_sb = sb.tile([1, C], F32, name="u1_sb", tag="u1_sb")
    nc.vector.tensor_copy(u1_sb, u1_ps)
    lhs2 = sb.tile([2, C], F32, name="lhs2", tag="lhs2")
    nc.sync.dma_start(lhs2[0:1, :], bqr)
    nc.sync.dma_start(lhs2[1:2, :], u1_sb)
    rhs2 = sb.tile([2, C], F32, name="rhs2", tag="rhs2")
    nc.vector.tensor_add(rhs2[0:1, :], u2_ps, nbkr_sb)
    nc.sync.dma_start(rhs2[1:2, :], bkr)

    # ---- T1 = G Wk^T via 3 f16 passes (hi*hi + hi*lo + lo*hi), f-outer ----
    t1_ps = [ps.tile([P, C], F32, name=f"t1ps{i}", tag=f"pa{i}") for i in range(CH)]
    for f in range(CH):
        for e in range(CH):
            nc.tensor.matmul(
                t1_ps[e], lhsT=gh[f][:, e * P : (e + 1) * P],
                rhs=wslice(wkh_sb, f, 0, C), start=f == 0, stop=False,
            )
    for f in range(CH):
        for e in range(CH):
            nc.tensor.matmul(
                t1_ps[e], lhsT=gh[f][:, e * P : (e + 1) * P],
                rhs=wslice(wkl_sb, f, 0, C), start=False, stop=False,
            )
    for f in range(CH):
        for e in range(CH):
            nc.tensor.matmul(
                t1_ps[e], lhsT=gl[f][:, e * P : (e + 1) * P],
                rhs=wslice(wkh_sb, f, 0, C), start=False, stop=f == CH - 1,
            )
    t1h = [sb.tile([P, C], F16, name=f"t1h{i}", tag=f"t1h{i}") for i in range(CH)]
    t1l = [sb.tile([P, C], F16, name=f"t1l{i}", tag=f"t1l{i}") for i in range(CH)]
    for e in range(CH):
        nc.scalar.copy(t1h[e], t1_ps[e])
        nc.vector.tensor_sub(t1l[e], t1_ps[e], t1h[e])

    # ---- logits = Wq T1 + rank-1 bias terms (fp32, PSUM-accumulated) ----
    l_ps = [ps.tile([P, C], F32, name=f"lps{i}", tag=f"pb{i}") for i in range(CH)]
    for c in range(CH):
        for e in range(CH):
            nc.tensor.matmul(
                l_ps[c], lhsT=wslice(wqh_sb, e, c * P, (c + 1) * P),
                rhs=t1h[e], start=e == 0, stop=False,
            )
        for e in range(CH):
            nc.tensor.matmul(
                l_ps[c], lhsT=wslice(wqh_sb, e, c * P, (c + 1) * P),
                rhs=t1l[e], start=False, stop=False,
            )
        for e in range(CH):
            nc.tensor.matmul(
                l_ps[c], lhsT=wslice(wql_sb, e, c * P, (c + 1) * P),
                rhs=t1h[e], start=False, stop=False,
            )
        nc.tensor.matmul(
            l_ps[c], lhsT=lhs2[:, c * P : (c + 1) * P], rhs=rhs2,
            start=False, stop=True,
        )

    # ---- softmax over rows of L ----
    w16_sb = [sb.tile([P, C], F16, name=f"w16_{i}", tag=f"w16_{i}") for i in range(CH)]
    for c in range(CH):
        negmx = sb.tile([P, 1], F32, name=f"negmx{c}", tag=f"negmx{c}")
        nc.vector.reduce_max(negmx, l_ps[c], axis=AX, negate=True)
        e_sb = sb.tile([P, C], F32, name="esb", tag="esb", bufs=2)
        ssum = sb.tile([P, 1], F32, name=f"ssum{c}", tag=f"ssum{c}")
        nc.scalar.activation(e_sb, l_ps[c], EXP, bias=negmx, scale=1.0, accum_out=ssum)
        rcp = sb.tile([P, 1], F32, name=f"rcp{c}", tag=f"rcp{c}")
        nc.vector.reciprocal(rcp, ssum)
        nc.vector.tensor_scalar_mul(w16_sb[c], e_sb, rcp)

    # ---- transpose softmax weights (fp16, PE) ----
    wt_sb = [sb.tile([P, C], F16, name=f"wtsb{j}", tag=f"wtsb{j}") for j in range(CH)]
    for j in range(CH):
        wt_ps = ps.tile([P, C], F16, name=f"wtps{j}", tag=f"pb{j}")
        for i in range(CH):
            nc.tensor.transpose(
                wt_ps[:, i * P : (i + 1) * P],
                w16_sb[i][:, j * P : (j + 1) * P],
                id16_sb,
            )
        nc.vector.tensor_copy(wt_sb[j], wt_ps)

    # ---- out = w v + x (fp16 matmuls, residual from fp16 x) ----
    def out_tile(nt, fine=False):
        for c in range(CH):
            o_ps = ps.tile([P, 512], F32, name=f"ops{c}", tag=f"pb{c}")
            for d in range(CH):
                nc.tensor.matmul(
                    o_ps,
                    lhsT=wt_sb[d][:, c * P : (c + 1) * P],
                    rhs=v_sb[d][:, nt * 512 : (nt + 1) * 512],
                    start=d == 0,
                    stop=d == CH - 1,
                )
            o_sb = so.tile([P, 512], F32, name="osb", tag="osb", bufs=4)
            pieces = ((0, 256), (256, 512)) if (fine and c == CH - 1) else ((0, 512),)
            for lo, hi in pieces:
                nc.vector.tensor_add(
                    o_sb[:, lo:hi], o_ps[:, lo:hi],
                    x16_sb[c][:, nt * 512 + lo : nt * 512 + hi],
                )
                nc.sync.dma_start(
                    out[c * P : (c + 1) * P, nt * 512 + lo : nt * 512 + hi],
                    o_sb[:, lo:hi],
                )

    out_tile(0)
    v_conv(7, "pa")
    for nt in range(1, NT):
        out_tile(nt)

    for pool in (so, st, sb, ps):
        pool.release()


def _build_nc(repeat=1):
    nc = bacc.Bacc(
        "TRN2",
        target_bir_lowering=False,
        debug=False,
        num_devices=B,
        enable_asserts=False,
    )
    io = {}
    dt = nc.dram_tensor
    io["xt16"] = dt("xt16", (HW, C), F16, kind="ExternalInput").ap()
    io["x16"] = dt("x16", (C, HW), F16, kind="ExternalInput").ap()
    io["wqh"] = dt("wqh", (C, C), F16, kind="ExternalInput").ap()
    io["wql"] = dt("wql", (C, C), F16, kind="ExternalInput").ap()
    io["wkh"] = dt("wkh", (C, C), F16, kind="ExternalInput").ap()
    io["wkl"] = dt("wkl", (C, C), F16, kind="ExternalInput").ap()
    io["wvt"] = dt("wvt", (C, C), F16, kind="ExternalInput").ap()
    io["bqr"] = dt("bqr", (1, C), F32, kind="ExternalInput").ap()
    io["bkr"] = dt("bkr", (1, C), F32, kind="ExternalInput").ap()
    io["nbkr"] = dt("nbkr", (1, C), F32, kind="ExternalInput").ap()
    io["bvc"] = dt("bvc", (C, 1), F32, kind="ExternalInput").ap()
    io["id16"] = dt("id16", (P, P), F16, kind="ExternalInput").ap()
    io["out"] = dt("out", (C, HW), F32, kind="ExternalOutput").ap()
    with tile.TileContext(nc) as tc:
        for _ in range(repeat):
            _body(tc, nc, io)
    nc.compile()
    return nc


_NC_CACHE = None


def get_nc():
    global _NC_CACHE
    if _NC_CACHE is None:
        _NC_CACHE = _build_nc()
    return _NC_CACHE


def prep_in_maps(x, wq, bq, wk, bk, wv, bv):
    """Host-side input prep: reshape/transpose/dtype casts only."""
    x = np.asarray(x, dtype=np.float32)
    X = x.reshape(B, C, HW)
    XT = np.ascontiguousarray(X.transpose(0, 2, 1))
    xt16 = XT.astype(np.float16)
    x16 = X.astype(np.float16)
    wqt = np.ascontiguousarray(np.asarray(wq, np.float32).T)
    wkt = np.ascontiguousarray(np.asarray(wk, np.float32).T)
    wqh = wqt.astype(np.float16)
    wql = (wqt - wqh.astype(np.float32)).astype(np.float16)
    wkh = wkt.astype(np.float16)
    wkl = (wkt - wkh.astype(np.float32)).astype(np.float16)
    wvt = np.ascontiguousarray(np.asarray(wv, np.float32).T).astype(np.float16)
    bqr = np.asarray(bq, np.float32).reshape(1, C)
    bkr = np.asarray(bk, np.float32).reshape(1, C)
    nbkr = (float(HW) * np.asarray(bk, np.float32)).reshape(1, C)
    bvc = np.asarray(bv, np.float32).reshape(C, 1)
    id16 = np.eye(P, dtype=np.float16)
    in_maps = []
    for b in range(B):
        in_maps.append(
            {
                "xt16": xt16[b],
                "x16": np.ascontiguousarray(x16[b]),
                "wqh": wqh,
                "wql": wql,
                "wkh": wkh,
                "wkl": wkl,
                "wvt": wvt,
                "bqr": bqr,
                "bkr": bkr,
                "nbkr": nbkr,
                "bvc": bvc,
                "id16": id16,
            }
        )
    return in_maps


def kernel(x, wq, bq, wk, bk, wv, bv):
    nc = get_nc()
    in_maps = prep_in_maps(x, wq, bq, wk, bk, wv, bv)
    res = run_bass_kernel_spmd(nc, in_maps, core_ids=list(range(B)))
    out = np.stack([res.results[b]["out"] for b in range(B)])
    return out.reshape(B, C, 64, 64).astype(np.float32)



# revision 4
# speedup vs baseline: 1.4789x; 1.4789x over previous
"""Channel-attention (nn_ChannelAttentionModule) Trainium2 kernel.

Math (per batch b, C=512, N=64*64=4096):
    X = x[b]  [C, N]
    q = Wq X + bq ; k = Wk X + bk ; v = Wv X + bv
    L = q k^T ; A = softmax(L, -1) ; out = A v + X

Restructure (all heavy matmuls fp16 single-pass, fp32 PSUM accum):
    L  = Wq G Wk^T + bq (Wk S + N bk)^T + (Wq S) bk^T
         with G = X X^T (Gram, symmetric; upper block-triangle + PE-transpose
         mirror) and S = X 1 (spatial row sums, free PE matmuls vs ones).
    A v = rcp (.) [ (E Wv) X + (E bv) 1^T ]
         with E = exp(L - rowmax), rcp = 1/rowsum(E).  Reassociating
         (E Wv) X kills the C^2 N v-conv entirely: M0 = E Wv is only C^3.
    out = Identity(o_ps * rcp + rcp*E bv) + X   (scalar-engine scale/bias,
         vector-engine residual add, fp16 store; host casts to fp32).

Sharding: pure data-parallel, one batch per NeuronCore (B=8, 8 cores).
"""

import numpy as np

import concourse.mybir as mybir
import concourse.tile as tile
from concourse import bacc
from concourse.bass_utils import run_bass_kernel_spmd

F32 = mybir.dt.float32
F16 = mybir.dt.float16
AX = mybir.AxisListType.X
EXP = mybir.ActivationFunctionType.Exp
IDENT = mybir.ActivationFunctionType.Identity

B = 8
C = 512
HW = 64 * 64
P = 128
CH = C // P  # 4 channel chunks
NG = 8  # xt granules (4 spatial tiles of 128 rows each)
# upper-triangle start per G row chunk
USTART = [0, 128, 256, 384]
# lower blocks (c,d) mirrored from upper gh[d][:, c-block]
MIRROR = [(1, 0), (2, 0), (2, 1), (3, 0), (3, 1), (3, 2)]
OTAGS = ["pa0", "pa1", "pa2", "pa3", "pb0", "pb1", "pb2", "pb3"]


def _body(tc, nc, io):
    xt16, x16 = io["xt16"], io["x16"]
    wqt, wkt, wv = io["wqt"], io["wkt"], io["wv"]
    bqh, bql, bk16, nbkr, bv16 = (
        io["bqh"], io["bql"], io["bk16"], io["nbkr"], io["bv16"],
    )
    id16, ones16, out = io["id16"], io["ones16"], io["out"]

    ps = tc.alloc_tile_pool(name="ps", bufs=1, space="PSUM")
    sb = tc.alloc_tile_pool(name="sb", bufs=1)

    # ---- persistent SBUF tiles ----
    x16_sb = [sb.tile([P, HW], F16, name=f"x16_{i}", tag=f"x16_{i}") for i in range(CH)]
    wqt_sb = sb.tile([P, CH * C], F16, name="wqt_sb", tag="wqt_sb")
    wkt_sb = sb.tile([P, CH * C], F16, name="wkt_sb", tag="wkt_sb")
    wv_sb = sb.tile([P, CH * C], F16, name="wv_sb", tag="wv_sb")
    bv_sb = sb.tile([P, CH], F16, name="bv_sb", tag="bv_sb")
    id16_sb = sb.tile([P, P], F16, name="id16_sb", tag="id16_sb")
    ones_sb = sb.tile([P, 1], F16, name="ones_sb", tag="ones_sb")
    ar_sb = [sb.tile([P, 4 * C], F16, name=f"ar{g}", tag=f"ar{g}") for g in range(NG)]
    gh = [sb.tile([P, C], F16, name=f"gh{i}", tag=f"gh{i}") for i in range(CH)]
    t1h = [sb.tile([P, C], F16, name=f"t1h{i}", tag=f"t1h{i}") for i in range(CH)]
    e16 = [sb.tile([P, C], F16, name=f"e16_{i}", tag=f"e16_{i}") for i in range(CH)]
    et_sb = [sb.tile([P, C], F16, name=f"et{j}", tag=f"et{j}") for j in range(CH)]
    m0t = [sb.tile([P, C], F16, name=f"m0t{j}", tag=f"m0t{j}") for j in range(CH)]
    s16 = [sb.tile([P, 1], F16, name=f"s16_{i}", tag=f"s16_{i}") for i in range(CH)]
    lhs4 = sb.tile([4, C], F16, name="lhs4", tag="lhs4")
    rhs4 = sb.tile([4, C], F16, name="rhs4", tag="rhs4")
    nbkr_sb = sb.tile([1, C], F32, name="nbkr_sb", tag="nbkr_sb")
    rhs2f = sb.tile([1, C], F32, name="rhs2f", tag="rhs2f")
    u1h_sb = sb.tile([1, C], F16, name="u1h_sb", tag="u1h_sb")
    rhs2h_sb = sb.tile([1, C], F16, name="rhs2h_sb", tag="rhs2h_sb")
    rhs2l_sb = sb.tile([1, C], F16, name="rhs2l_sb", tag="rhs2l_sb")

    def wslice(tile_, e, lo, hi):
        return tile_[:, e * C + lo : e * C + hi]

    # ---- DMA schedule (SP queue order == DMA device order) ----
    xtr3 = xt16.rearrange("(g t p) c -> g p t c", p=P, t=4)
    nc.sync.dma_start(
        ar_sb[0][:, 0 : 2 * C].rearrange("p (t c) -> p t c", t=2), xtr3[0][:, 0:2]
    )
    nc.sync.dma_start(
        ar_sb[0][:, 2 * C :].rearrange("p (t c) -> p t c", t=2), xtr3[0][:, 2:4]
    )
    for g in range(1, NG):
        nc.sync.dma_start(ar_sb[g].rearrange("p (t c) -> p t c", t=4), xtr3[g])
    nc.sync.dma_start(id16_sb, id16)
    nc.sync.dma_start(ones_sb, ones16)
    nc.sync.dma_start(
        wkt_sb.rearrange("p (e c) -> p e c", e=CH),
        wkt.rearrange("(e p) c -> p e c", p=P),
    )
    nc.sync.dma_start(
        wqt_sb.rearrange("p (e c) -> p e c", e=CH),
        wqt.rearrange("(e p) c -> p e c", p=P),
    )
    nc.sync.dma_start(lhs4[0:1, :], bqh)
    nc.sync.dma_start(lhs4[1:2, :], bql)
    nc.sync.dma_start(lhs4[2:3, :], bqh)
    nc.sync.dma_start(rhs4[3:4, :], bk16)
    nc.sync.dma_start(nbkr_sb, nbkr)
    nc.sync.dma_start(
        wv_sb.rearrange("p (e c) -> p e c", e=CH),
        wv.rearrange("(e p) c -> p e c", p=P),
    )
    nc.sync.dma_start(
        bv_sb.rearrange("p (e o) -> p e o", e=CH),
        bv16.rearrange("(e p) o -> p e o", p=P),
    )
    for c in range(CH):
        nc.sync.dma_start(x16_sb[c], x16[c * P : (c + 1) * P, :])

    # ---- G = X X^T (upper block-triangle) + S = X 1, accumulated on PE ----
    g_ps = [ps.tile([P, C], F32, name=f"gps{i}", tag=f"pa{i}") for i in range(CH)]
    s_ps = ps.tile([P, CH], F32, name="s_ps", tag="pb0")

    for g2 in range(NG):
        ar4 = ar_sb[g2]
        for t in range(4):
            n = g2 * 4 + t
            first, last = n == 0, n == 4 * NG - 1
            for c in range(CH):
                u = USTART[c]
                nc.tensor.matmul(
                    g_ps[c][:, u:],
                    lhsT=ar4[:, t * C + c * P : t * C + (c + 1) * P],
                    rhs=ar4[:, t * C + u : (t + 1) * C],
                    start=first,
                    stop=last,
                )
            for c in range(CH):
                nc.tensor.matmul(
                    s_ps[:, c : c + 1],
                    lhsT=ar4[:, t * C + c * P : t * C + (c + 1) * P],
                    rhs=ones_sb,
                    start=first,
                    stop=last,
                )

    # ---- gh = f16(G); mirror lower blocks via fp16 PE transposes ----
    for c in range(CH):
        nc.scalar.copy(gh[c][:, USTART[c] :], g_ps[c][:, USTART[c] :])
    for i in range(CH):
        nc.vector.tensor_copy(s16[i], s_ps[:, i : i + 1])
    for idx, (c, d) in enumerate(MIRROR):
        tb = ps.tile(
            [P, P], F16, name="tb", tag="pb3" if idx % 2 == 0 else "pb0"
        )
        nc.tensor.transpose(tb, gh[d][:, c * P : (c + 1) * P], id16_sb)
        nc.scalar.copy(gh[c][:, d * P : (d + 1) * P], tb)

    # ---- u1 = (Wq S)^T, u2 = (Wk S)^T; rank-2 bias factors (fp16 hi/lo) ----
    u1_ps = ps.tile([1, C], F32, name="u1_ps", tag="pb1")
    u2_ps = ps.tile([1, C], F32, name="u2_ps", tag="pb2")
    for e in range(CH):
        nc.tensor.matmul(
            u1_ps, lhsT=s16[e], rhs=wslice(wqt_sb, e, 0, C),
            start=e == 0, stop=e == CH - 1,
        )
    for e in range(CH):
        nc.tensor.matmul(
            u2_ps, lhsT=s16[e], rhs=wslice(wkt_sb, e, 0, C),
            start=e == 0, stop=e == CH - 1,
        )
    # engines may only write partition offset 0 here; place rows via DMA
    nc.scalar.copy(u1h_sb, u1_ps)
    nc.vector.tensor_add(rhs2f, u2_ps, nbkr_sb)
    nc.scalar.copy(rhs2h_sb, rhs2f)
    nc.vector.tensor_sub(rhs2l_sb, rhs2f, rhs2h_sb)
    nc.sync.dma_start(lhs4[3:4, :], u1h_sb)
    nc.sync.dma_start(rhs4[0:1, :], rhs2h_sb)
    nc.sync.dma_start(rhs4[1:2, :], rhs2h_sb)
    nc.sync.dma_start(rhs4[2:3, :], rhs2l_sb)

    # ---- T1 = G Wk^T (single fp16 pass; lhsT = G blocks via symmetry) ----
    t1_ps = [ps.tile([P, C], F32, name=f"t1ps{e}", tag=f"pa{e}") for e in range(CH)]
    for e in range(CH):
        for f in range(CH):
            nc.tensor.matmul(
                t1_ps[e], lhsT=gh[f][:, e * P : (e + 1) * P],
                rhs=wslice(wkt_sb, f, 0, C), start=f == 0, stop=f == CH - 1,
            )
        nc.scalar.copy(t1h[e], t1_ps[e])

    # ---- logits = Wq T1 + rank-2 (K=4 fp16); softmax row stats ----
    l_ps = [ps.tile([P, C], F32, name=f"lps{c}", tag=f"pb{c}") for c in range(CH)]
    negmx = [sb.tile([P, 1], F32, name=f"negmx{c}", tag=f"negmx{c}") for c in range(CH)]
    ssum = [sb.tile([P, 1], F32, name=f"ssum{c}", tag=f"ssum{c}") for c in range(CH)]
    rcp = [sb.tile([P, 1], F32, name=f"rcp{c}", tag=f"rcp{c}") for c in range(CH)]
    for c in range(CH):
        nc.tensor.matmul(
            l_ps[c], lhsT=lhs4[:, c * P : (c + 1) * P], rhs=rhs4,
            start=True, stop=False,
        )
        for e in range(CH):
            nc.tensor.matmul(
                l_ps[c], lhsT=wslice(wqt_sb, e, c * P, (c + 1) * P), rhs=t1h[e],
                start=False, stop=e == CH - 1,
            )
        nc.vector.reduce_max(negmx[c], l_ps[c], axis=AX, negate=True)
        nc.scalar.activation(
            e16[c], l_ps[c], EXP, bias=negmx[c], scale=1.0, accum_out=ssum[c]
        )
        nc.vector.reciprocal(rcp[c], ssum[c])

    # ---- E^T via fp16 PE transposes ----
    et_ps = [ps.tile([P, C], F16, name=f"etps{j}", tag=f"pb{j}") for j in range(CH)]
    for c in range(CH):
        for j in range(CH):
            nc.tensor.transpose(
                et_ps[j][:, c * P : (c + 1) * P],
                e16[c][:, j * P : (j + 1) * P],
                id16_sb,
            )
    for j in range(CH):
        nc.vector.tensor_copy(et_sb[j], et_ps[j])

    # ---- r0 = E bv; rr = rcp * r0 ----
    rr = [sb.tile([P, 1], F32, name=f"rr{c}", tag=f"rr{c}") for c in range(CH)]
    for c in range(CH):
        r0_ps = ps.tile([P, 1], F32, name=f"r0ps{c}", tag=f"pb{c}")
        for d in range(CH):
            nc.tensor.matmul(
                r0_ps, lhsT=et_sb[d][:, c * P : (c + 1) * P],
                rhs=bv_sb[:, d : d + 1], start=d == 0, stop=d == CH - 1,
            )
        nc.vector.tensor_scalar_mul(rr[c], r0_ps, rcp[c])

    # ---- M0^T = Wv^T E^T (fp16, C^3 only) ----
    m0t_ps = [ps.tile([P, C], F32, name=f"m0tps{d}", tag=f"pa{d}") for d in range(CH)]
    for d in range(CH):
        for e in range(CH):
            nc.tensor.matmul(
                m0t_ps[d], lhsT=wslice(wv_sb, e, d * P, (d + 1) * P),
                rhs=et_sb[e], start=e == 0, stop=e == CH - 1,
            )
        nc.scalar.copy(m0t[d], m0t_ps[d])

    # ---- out = rcp*(M0^T^T X) + rr + X, fp16 store per row-block ----
    for c in range(CH):
        o16 = sb.tile([P, HW], F16, name="o16", tag="o16", bufs=2)
        for nt in range(HW // 512):
            o_ps = ps.tile(
                [P, 512], F32, name="o_ps", tag=OTAGS[(c * 8 + nt) % 8]
            )
            for d in range(CH):
                nc.tensor.matmul(
                    o_ps,
                    lhsT=m0t[d][:, c * P : (c + 1) * P],
                    rhs=x16_sb[d][:, nt * 512 : (nt + 1) * 512],
                    start=d == 0,
                    stop=d == CH - 1,
                )
            t_sb = sb.tile([P, 512], F16, name="t_sb", tag="t_sb", bufs=4)
            nc.scalar.activation(t_sb, o_ps, IDENT, bias=rr[c], scale=rcp[c])
            nc.vector.tensor_add(
                o16[:, nt * 512 : (nt + 1) * 512],
                t_sb,
                x16_sb[c][:, nt * 512 : (nt + 1) * 512],
            )
        nc.gpsimd.dma_start(out[c * P : (c + 1) * P, :], o16)

    for pool in (sb, ps):
        pool.release()


def _build_nc(repeat=1):
    nc = bacc.Bacc(
        "TRN2",
        target_bir_lowering=False,
        debug=False,
        num_devices=B,
        enable_asserts=False,
    )
    io = {}
    dt = nc.dram_tensor
    io["xt16"] = dt("xt16", (HW, C), F16, kind="ExternalInput").ap()
    io["x16"] = dt("x16", (C, HW), F16, kind="ExternalInput").ap()
    io["wqt"] = dt("wqt", (C, C), F16, kind="ExternalInput").ap()
    io["wkt"] = dt("wkt", (C, C), F16, kind="ExternalInput").ap()
    io["wv"] = dt("wv", (C, C), F16, kind="ExternalInput").ap()
    io["bqh"] = dt("bqh", (1, C), F16, kind="ExternalInput").ap()
    io["bql"] = dt("bql", (1, C), F16, kind="ExternalInput").ap()
    io["bk16"] = dt("bk16", (1, C), F16, kind="ExternalInput").ap()
    io["nbkr"] = dt("nbkr", (1, C), F32, kind="ExternalInput").ap()
    io["bv16"] = dt("bv16", (C, 1), F16, kind="ExternalInput").ap()
    io["id16"] = dt("id16", (P, P), F16, kind="ExternalInput").ap()
    io["ones16"] = dt("ones16", (P, 1), F16, kind="ExternalInput").ap()
    io["out"] = dt("out", (C, HW), F16, kind="ExternalOutput").ap()
    with tile.TileContext(nc) as tc:
        for _ in range(repeat):
            _body(tc, nc, io)
    nc.compile()
    return nc


_NC_CACHE = None


def get_nc():
    global _NC_CACHE
    if _NC_CACHE is None:
        _NC_CACHE = _build_nc()
    return _NC_CACHE


def prep_in_maps(x, wq, bq, wk, bk, wv, bv):
    """Host-side input prep: reshape/transpose/dtype casts only."""
    x = np.asarray(x, dtype=np.float32)
    X = x.reshape(B, C, HW)
    xt16 = np.ascontiguousarray(X.transpose(0, 2, 1)).astype(np.float16)
    x16 = X.astype(np.float16)
    wqt = np.ascontiguousarray(np.asarray(wq, np.float32).T).astype(np.float16)
    wkt = np.ascontiguousarray(np.asarray(wk, np.float32).T).astype(np.float16)
    wv16 = np.asarray(wv, np.float32).astype(np.float16)
    bq32 = np.asarray(bq, np.float32).reshape(1, C)
    bqh = bq32.astype(np.float16)
    bql = (bq32 - bqh.astype(np.float32)).astype(np.float16)
    bk16 = np.asarray(bk, np.float32).reshape(1, C).astype(np.float16)
    nbkr = (float(HW) * np.asarray(bk, np.float32)).reshape(1, C)
    bv16 = np.asarray(bv, np.float32).reshape(C, 1).astype(np.float16)
    id16 = np.eye(P, dtype=np.float16)
    ones16 = np.ones((P, 1), dtype=np.float16)
    in_maps = []
    for b in range(B):
        in_maps.append(
            {
                "xt16": xt16[b],
                "x16": np.ascontiguousarray(x16[b]),
                "wqt": wqt,
                "wkt": wkt,
                "wv": wv16,
                "bqh": bqh,
                "bql": bql,
                "bk16": bk16,
                "nbkr": nbkr,
                "bv16": bv16,
                "id16": id16,
                "ones16": ones16,
            }
        )
    return in_maps


def kernel(x, wq, bq, wk, bk, wv, bv):
    nc = get_nc()
    in_maps = prep_in_maps(x, wq, bq, wk, bk, wv, bv)
    res = run_bass_kernel_spmd(nc, in_maps, core_ids=list(range(B)))
    out = np.stack([res.results[b]["out"] for b in range(B)])
    return out.reshape(B, C, 64, 64).astype(np.float32)


# revision 14
# speedup vs baseline: 1.4896x; 1.0072x over previous
"""Channel-attention (nn_ChannelAttentionModule) Trainium2 kernel.

Math (per batch b, C=512, N=64*64=4096):
    X = x[b]  [C, N]
    q = Wq X + bq ; k = Wk X + bk ; v = Wv X + bv
    L = q k^T ; A = softmax(L, -1) ; out = A v + X

Restructure (all heavy matmuls fp16 single-pass, fp32 PSUM accum):
    L  = Wq G Wk^T + bq (Wk S + N bk)^T + (Wq S) bk^T
         with G = X X^T (Gram, symmetric; upper block-triangle + PE-transpose
         mirror) and S = X 1 (spatial row sums, free PE matmuls vs ones).
    A v = rcp (.) [ (E Wv) X + (E bv) 1^T ]
         with E = exp(L - rowmax), rcp = 1/rowsum(E).  Reassociating
         (E Wv) X kills the C^2 N v-conv entirely: M0 = E Wv is only C^3.
    out = Identity(o_ps * rcp + rcp*E bv) + X   (scalar-engine scale/bias,
         vector-engine residual add, fp16 store; host casts to fp32).

Sharding: pure data-parallel, one batch per NeuronCore (B=8, 8 cores).
"""

import numpy as np

import concourse.mybir as mybir
import concourse.tile as tile
from concourse import bacc
from concourse.bass_utils import run_bass_kernel_spmd

F32 = mybir.dt.float32
F32R = mybir.dt.float32r
F16 = mybir.dt.float16
AX = mybir.AxisListType.X
EXP = mybir.ActivationFunctionType.Exp
IDENT = mybir.ActivationFunctionType.Identity

B = 8
C = 512
HW = 64 * 64
P = 128
CH = C // P  # 4 channel chunks
NG = 8  # xt granules (4 spatial tiles of 128 rows each)
# upper-triangle start per G row chunk
USTART = [0, 128, 256, 384]
# lower blocks (c,d) mirrored from upper gh[d][:, c-block]
MIRROR = [(1, 0), (2, 0), (2, 1), (3, 0), (3, 1), (3, 2)]
OTAGS = ["pa0", "pa1", "pa2", "pa3", "pb0", "pb1", "pb2", "pb3"]


def _body(tc, nc, io):
    xt16, x16 = io["xt16"], io["x16"]
    wqt, wkt, wv = io["wqt"], io["wkt"], io["wv"]
    bqr, bkr, nbkr, bv16 = io["bqr"], io["bkr"], io["nbkr"], io["bv16"]
    id16, id32, ones16, out = io["id16"], io["id32"], io["ones16"], io["out"]

    ps = tc.alloc_tile_pool(name="ps", bufs=1, space="PSUM")
    sb = tc.alloc_tile_pool(name="sb", bufs=1)

    # ---- persistent SBUF tiles ----
    x16_sb = [sb.tile([P, HW], F16, name=f"x16_{i}", tag=f"x16_{i}") for i in range(CH)]
    wqt_sb = sb.tile([P, CH * C], F32R, name="wqt_sb", tag="wqt_sb")
    wkt_sb = sb.tile([P, CH * C], F32R, name="wkt_sb", tag="wkt_sb")
    wv_sb = sb.tile([P, CH * C], F16, name="wv_sb", tag="wv_sb")
    bv_sb = sb.tile([P, CH], F16, name="bv_sb", tag="bv_sb")
    id16_sb = sb.tile([P, P], F16, name="id16_sb", tag="id16_sb")
    id32_sb = sb.tile([P, P], F32R, name="id32_sb", tag="id32_sb")
    ones_sb = sb.tile([P, 1], F16, name="ones_sb", tag="ones_sb")
    ar_sb = [sb.tile([P, 4 * C], F16, name=f"ar{g}", tag=f"ar{g}") for g in range(NG)]
    gf = [sb.tile([P, C], F32R, name=f"gf{i}", tag=f"gf{i}") for i in range(CH)]
    t1f = [sb.tile([P, C], F32R, name=f"t1f{i}", tag=f"t1f{i}") for i in range(CH)]
    e16 = [sb.tile([P, C], F16, name=f"e16_{i}", tag=f"e16_{i}") for i in range(CH)]
    et_sb = [sb.tile([P, C], F16, name=f"et{j}", tag=f"et{j}") for j in range(CH)]
    m0t = [sb.tile([P, C], F16, name=f"m0t{j}", tag=f"m0t{j}") for j in range(CH)]
    s32 = [sb.tile([P, 1], F32R, name=f"s32_{i}", tag=f"s32_{i}") for i in range(CH)]
    lhs2 = sb.tile([2, C], F32R, name="lhs2", tag="lhs2")
    rhs2 = sb.tile([2, C], F32R, name="rhs2", tag="rhs2")
    nbkr_sb = sb.tile([1, C], F32, name="nbkr_sb", tag="nbkr_sb")
    u1f_sb = sb.tile([1, C], F32R, name="u1f_sb", tag="u1f_sb")

    def wslice(tile_, e, lo, hi):
        return tile_[:, e * C + lo : e * C + hi]

    # ---- DMA schedule (SP queue order == DMA device order) ----
    xtr3 = xt16.rearrange("(g t p) c -> g p t c", p=P, t=4)
    nc.sync.dma_start(
        ar_sb[0][:, 0 : 2 * C].rearrange("p (t c) -> p t c", t=2), xtr3[0][:, 0:2]
    )
    nc.sync.dma_start(
        ar_sb[0][:, 2 * C :].rearrange("p (t c) -> p t c", t=2), xtr3[0][:, 2:4]
    )
    for g in range(1, NG):
        nc.sync.dma_start(ar_sb[g].rearrange("p (t c) -> p t c", t=4), xtr3[g])
    nc.sync.dma_start(id16_sb, id16)
    nc.sync.dma_start(id32_sb, id32)
    nc.sync.dma_start(ones_sb, ones16)
    nc.sync.dma_start(
        wkt_sb.rearrange("p (e c) -> p e c", e=CH),
        wkt.rearrange("(e p) c -> p e c", p=P),
    )
    nc.sync.dma_start(
        wqt_sb.rearrange("p (e c) -> p e c", e=CH),
        wqt.rearrange("(e p) c -> p e c", p=P),
    )
    nc.sync.dma_start(lhs2[0:1, :], bqr)
    nc.sync.dma_start(rhs2[1:2, :], bkr)
    nc.sync.dma_start(nbkr_sb, nbkr)
    nc.sync.dma_start(
        wv_sb.rearrange("p (e c) -> p e c", e=CH),
        wv.rearrange("(e p) c -> p e c", p=P),
    )
    nc.sync.dma_start(
        bv_sb.rearrange("p (e o) -> p e o", e=CH),
        bv16.rearrange("(e p) o -> p e o", p=P),
    )
    for c in range(CH):
        nc.sync.dma_start(x16_sb[c], x16[c * P : (c + 1) * P, :])

    # ---- G = X X^T (upper block-triangle) + S = X 1, accumulated on PE ----
    g_ps = [ps.tile([P, C], F32, name=f"gps{i}", tag=f"pa{i}") for i in range(CH)]
    s_ps = ps.tile([P, CH], F32, name="s_ps", tag="pb0")

    for g2 in range(NG):
        ar4 = ar_sb[g2]
        for t in range(4):
            n = g2 * 4 + t
            first, last = n == 0, n == 4 * NG - 1
            for c in range(CH):
                u = USTART[c]
                nc.tensor.matmul(
                    g_ps[c][:, u:],
                    lhsT=ar4[:, t * C + c * P : t * C + (c + 1) * P],
                    rhs=ar4[:, t * C + u : (t + 1) * C],
                    start=first,
                    stop=last,
                )
            for c in range(CH):
                nc.tensor.matmul(
                    s_ps[:, c : c + 1],
                    lhsT=ar4[:, t * C + c * P : t * C + (c + 1) * P],
                    rhs=ones_sb,
                    start=first,
                    stop=last,
                )

    # ---- gf = fp32 G in SBUF; mirror lower blocks via fp32 PE transposes ----
    for c in range(CH):
        nc.scalar.copy(gf[c][:, USTART[c] :], g_ps[c][:, USTART[c] :])
    for i in range(CH):
        nc.vector.tensor_copy(s32[i], s_ps[:, i : i + 1])
    for idx, (c, d) in enumerate(MIRROR):
        tb = ps.tile(
            [P, P], F32R, name="tb", tag="pb3" if idx % 2 == 0 else "pb0"
        )
        nc.tensor.transpose(tb, gf[d][:, c * P : (c + 1) * P], id32_sb)
        nc.scalar.copy(gf[c][:, d * P : (d + 1) * P], tb)

    # ---- u1 = (Wq S)^T, u2 = (Wk S)^T (fp32r); rank-2 bias factors ----
    u1_ps = ps.tile([1, C], F32, name="u1_ps", tag="pb1")
    u2_ps = ps.tile([1, C], F32, name="u2_ps", tag="pb2")
    for e in range(CH):
        nc.tensor.matmul(
            u1_ps, lhsT=s32[e], rhs=wslice(wqt_sb, e, 0, C),
            start=e == 0, stop=e == CH - 1,
        )
    for e in range(CH):
        nc.tensor.matmul(
            u2_ps, lhsT=s32[e], rhs=wslice(wkt_sb, e, 0, C),
            start=e == 0, stop=e == CH - 1,
        )
    # engines may only write partition offset 0; place row 1 via DMA
    nc.scalar.copy(u1f_sb, u1_ps)
    nc.vector.tensor_add(rhs2[0:1, :], u2_ps, nbkr_sb)
    nc.sync.dma_start(lhs2[1:2, :], u1f_sb)

    # ---- T1 = G Wk^T (single fp32r pass; lhsT = G blocks via symmetry) ----
    t1_ps = [ps.tile([P, C], F32, name=f"t1ps{e}", tag=f"pa{e}") for e in range(CH)]
    for e in range(CH):
        for f in range(CH):
            nc.tensor.matmul(
                t1_ps[e], lhsT=gf[f][:, e * P : (e + 1) * P],
                rhs=wslice(wkt_sb, f, 0, C), start=f == 0, stop=f == CH - 1,
            )
        nc.scalar.copy(t1f[e], t1_ps[e])

    # ---- logits = Wq T1 + rank-2 (all fp32r); softmax row stats ----
    l_ps = [ps.tile([P, C], F32, name=f"lps{c}", tag=f"pb{c}") for c in range(CH)]
    negmx = [sb.tile([P, 1], F32, name=f"negmx{c}", tag=f"negmx{c}") for c in range(CH)]
    ssum = [sb.tile([P, 1], F32, name=f"ssum{c}", tag=f"ssum{c}") for c in range(CH)]
    rcp = [sb.tile([P, 1], F32, name=f"rcp{c}", tag=f"rcp{c}") for c in range(CH)]
    for c in range(CH):
        nc.tensor.matmul(
            l_ps[c], lhsT=lhs2[:, c * P : (c + 1) * P], rhs=rhs2,
            start=True, stop=False,
        )
        for e in range(CH):
            nc.tensor.matmul(
                l_ps[c], lhsT=wslice(wqt_sb, e, c * P, (c + 1) * P),
                rhs=t1f[e], start=False, stop=e == CH - 1,
            )
        nc.vector.reduce_max(negmx[c], l_ps[c], axis=AX, negate=True)
        nc.scalar.activation(
            e16[c], l_ps[c], EXP, bias=negmx[c], scale=1.0, accum_out=ssum[c]
        )
        nc.vector.reciprocal(rcp[c], ssum[c])

    # ---- E^T via fp16 PE transposes ----
    et_ps = [ps.tile([P, C], F16, name=f"etps{j}", tag=f"pb{j}") for j in range(CH)]
    for c in range(CH):
        for j in range(CH):
            nc.tensor.transpose(
                et_ps[j][:, c * P : (c + 1) * P],
                e16[c][:, j * P : (j + 1) * P],
                id16_sb,
            )
    for j in range(CH):
        nc.vector.tensor_copy(et_sb[j], et_ps[j])

    # ---- r0 = E bv; rr = rcp * r0 ----
    rr = [sb.tile([P, 1], F32, name=f"rr{c}", tag=f"rr{c}") for c in range(CH)]
    for c in range(CH):
        r0_ps = ps.tile([P, 1], F32, name=f"r0ps{c}", tag=f"pb{c}")
        for d in range(CH):
            nc.tensor.matmul(
                r0_ps, lhsT=et_sb[d][:, c * P : (c + 1) * P],
                rhs=bv_sb[:, d : d + 1], start=d == 0, stop=d == CH - 1,
            )
        nc.vector.tensor_scalar_mul(rr[c], r0_ps, rcp[c])

    # ---- M0^T = Wv^T E^T (fp16, C^3 only) ----
    m0t_ps = [ps.tile([P, C], F32, name=f"m0tps{d}", tag=f"pa{d}") for d in range(CH)]
    for d in range(CH):
        for e in range(CH):
            nc.tensor.matmul(
                m0t_ps[d], lhsT=wslice(wv_sb, e, d * P, (d + 1) * P),
                rhs=et_sb[e], start=e == 0, stop=e == CH - 1,
            )
        nc.scalar.copy(m0t[d], m0t_ps[d])

    # ---- out = rcp*(M0^T^T X) + rr + X, fp16 store per row-block ----
    for c in range(CH):
        o16 = sb.tile([P, HW], F16, name="o16", tag="o16", bufs=2)
        for nt in range(HW // 512):
            o_ps = ps.tile(
                [P, 512], F32, name="o_ps", tag=OTAGS[(c * 8 + nt) % 8]
            )
            for d in range(CH):
                nc.tensor.matmul(
                    o_ps,
                    lhsT=m0t[d][:, c * P : (c + 1) * P],
                    rhs=x16_sb[d][:, nt * 512 : (nt + 1) * 512],
                    start=d == 0,
                    stop=d == CH - 1,
                )
            t_sb = sb.tile([P, 512], F16, name="t_sb", tag="t_sb", bufs=4)
            nc.scalar.activation(t_sb, o_ps, IDENT, bias=rr[c], scale=rcp[c])
            nc.vector.tensor_add(
                o16[:, nt * 512 : (nt + 1) * 512],
                t_sb,
                x16_sb[c][:, nt * 512 : (nt + 1) * 512],
            )
        nc.gpsimd.dma_start(out[c * P : (c + 1) * P, :], o16)

    for pool in (sb, ps):
        pool.release()


def _build_nc(repeat=1):
    nc = bacc.Bacc(
        "TRN2",
        target_bir_lowering=False,
        debug=False,
        num_devices=B,
        enable_asserts=False,
    )
    io = {}
    dt = nc.dram_tensor
    io["xt16"] = dt("xt16", (HW, C), F16, kind="ExternalInput").ap()
    io["x16"] = dt("x16", (C, HW), F16, kind="ExternalInput").ap()
    io["wqt"] = dt("wqt", (C, C), F32R, kind="ExternalInput").ap()
    io["wkt"] = dt("wkt", (C, C), F32R, kind="ExternalInput").ap()
    io["wv"] = dt("wv", (C, C), F16, kind="ExternalInput").ap()
    io["bqr"] = dt("bqr", (1, C), F32R, kind="ExternalInput").ap()
    io["bkr"] = dt("bkr", (1, C), F32R, kind="ExternalInput").ap()
    io["nbkr"] = dt("nbkr", (1, C), F32, kind="ExternalInput").ap()
    io["bv16"] = dt("bv16", (C, 1), F16, kind="ExternalInput").ap()
    io["id16"] = dt("id16", (P, P), F16, kind="ExternalInput").ap()
    io["id32"] = dt("id32", (P, P), F32R, kind="ExternalInput").ap()
    io["ones16"] = dt("ones16", (P, 1), F16, kind="ExternalInput").ap()
    io["out"] = dt("out", (C, HW), F16, kind="ExternalOutput").ap()
    with tile.TileContext(nc) as tc:
        for _ in range(repeat):
            _body(tc, nc, io)
    nc.compile()
    return nc


_NC_CACHE = None


def get_nc():
    global _NC_CACHE
    if _NC_CACHE is None:
        _NC_CACHE = _build_nc()
    return _NC_CACHE


def prep_in_maps(x, wq, bq, wk, bk, wv, bv):
    """Host-side input prep: reshape/transpose/dtype casts only."""
    x = np.asarray(x, dtype=np.float32)
    X = x.reshape(B, C, HW)
    xt16 = np.ascontiguousarray(X.transpose(0, 2, 1)).astype(np.float16)
    x16 = X.astype(np.float16)
    wqt = np.ascontiguousarray(np.asarray(wq, np.float32).T)
    wkt = np.ascontiguousarray(np.asarray(wk, np.float32).T)
    wv16 = np.asarray(wv, np.float32).astype(np.float16)
    bqr = np.asarray(bq, np.float32).reshape(1, C)
    bkr = np.asarray(bk, np.float32).reshape(1, C)
    nbkr = (float(HW) * np.asarray(bk, np.float32)).reshape(1, C)
    bv16 = np.asarray(bv, np.float32).reshape(C, 1).astype(np.float16)
    id16 = np.eye(P, dtype=np.float16)
    id32 = np.eye(P, dtype=np.float32)
    ones16 = np.ones((P, 1), dtype=np.float16)
    in_maps = []
    for b in range(B):
        in_maps.append(
            {
                "xt16": xt16[b],
                "x16": np.ascontiguousarray(x16[b]),
                "wqt": wqt,
                "wkt": wkt,
                "wv": wv16,
                "bqr": bqr,
                "bkr": bkr,
                "nbkr": nbkr,
                "bv16": bv16,
                "id16": id16,
                "id32": id32,
                "ones16": ones16,
            }
        )
    return in_maps


def kernel(x, wq, bq, wk, bk, wv, bv):
    nc = get_nc()
    in_maps = prep_in_maps(x, wq, bq, wk, bk, wv, bv)
    res = run_bass_kernel_spmd(nc, in_maps, core_ids=list(range(B)))
    out = np.stack([res.results[b]["out"] for b in range(B)])
    return out.reshape(B, C, 64, 64).astype(np.float32)
